# revision 21
# baseline (speedup 1.0000x reference)
"""Trainium2 Bass kernel for nn_Decoder (GRU decoder, B=64, T_FC=48, C=4096, HID=64).

Strategy (v2)
-------------
Data-parallel over batch: 8 cores x 8 batch rows -> 32768 independent GRU
"columns" per core.  Columns are processed in GROUPS of 1024: chunk A
(cols 0:512) occupies partitions 0:64, chunk B (cols 512:1024) partitions
64:128, so every DVE/ACT op runs with all 128 partitions busy.

Host algebra folds fc_in and the autoregressive x_prev feedback into the
gate weights (t>=1):
    G = W_ih @ W_in                       [192, 4]
    pre_g   = (W_hh_g + G_g0 wo^T) h + G_g,1:4 xt + bias_g     (g in r,z)
    i_n     = (G_n0 wo^T) h + G_n,1:4 xt + bias_n
    h_n     = W_hh_n h            (+ b_hh_n via scalar_tensor_tensor)
    n = tanh(i_n + r*h_n);  h' = n + z*(h - n);  pred = wo @ h' (+ b_out host)

Per group-step: 9 back-to-back N=512 bf16 matmuls -- ALL K=128 (any
K<128 matmul runs ~2x slower on this HW and poisons its neighbours, so
the K=9 x-side lhsTs are embedded at partition rows 9q:9q+9 of a
[128,128] zero-padded weight, against a [128,1024] tile that packs 12
timesteps of xt data in the partition dim).  Block-diagonal K=128
h-side matmuls; an identity-matmul accumulates r*h_n into the i_n psum
(gate banks are reused phase1->phase2); a sparse-lhsT pred matmul parks
4 steps x 8 groups of predictions in one double-buffered psum bank.
Per group-step ACT: 1 sigmoid [128,1024] + 1 tanh [128,512]; DVE: one
fused scalar_tensor_tensor (b_hh_n add + r mult) and 3 tensor_tensor
update ops at [128,1024] per double-group.  Final balance: PE 92%,
ACT 99%, DVE 79% busy.
"""

import os

import numpy as np

import concourse.bass as bass
import concourse.mybir as mybir
import concourse.tile as tile
from concourse import bacc
from concourse.bass_utils import run_bass_kernel_spmd

F32 = mybir.dt.float32
BF16 = mybir.dt.bfloat16
AF = mybir.ActivationFunctionType
ALU = mybir.AluOpType

B, T_HIST, T_FC, C, F_IN, HID = 64, 24, 48, 4096, 8, 64
N_CORES = 8
B_LOC = B // N_CORES
NCOLS = B_LOC * C          # 32768 columns per core
NG = 32                    # groups of 1024 columns
ND = 16                    # double-groups
W_GROUPS = 16              # groups per window (8 double-groups)
XQ = 12                    # xt steps packed per [128,1024] tile

_BUILT = {}
LAST_RESULTS = None  # BassKernelResults of the most recent run (for test.py)

W_SHAPES = {
    "TRH0": [128, 128], "TRH1": [128, 128],
    "TZH0": [128, 128], "TZH1": [128, 128],
    "TIH1": [128, 128], "THH": [128, 128], "ID128": [128, 128],
    "XR0P": [128, 128], "XZ0P": [128, 128], "XI0P": [128, 128],
    "XRP": [128, 12 * 128], "XZP": [128, 12 * 128], "XIP": [128, 12 * 128],
    "PW": [128, 64 * 128],
    "BHHN": [128, 1],
}


def _build():
    key = "v2"
    if key in _BUILT:
        return _BUILT[key]

    nc = bacc.Bacc("TRN2", target_bir_lowering=False, debug=False,
                   num_devices=N_CORES)

    d_xtd = nc.dram_tensor("XTD", [ND, 4, 128, 1024], BF16,
                           kind="ExternalInput").ap()
    d_ht = nc.dram_tensor("HT", [ND, 128, 1024], BF16,
                          kind="ExternalInput").ap()
    d_w = {name: nc.dram_tensor(name, shape,
                                F32 if name == "BHHN" else BF16,
                                kind="ExternalInput").ap()
           for name, shape in W_SHAPES.items()}
    # preds: [window, 128, 12*512]; row = 8*gi + 2*(t%4) + chunk
    d_out = nc.dram_tensor("OUT", [NG // W_GROUPS, 128, 6144], BF16,
                           kind="ExternalOutput").ap()

    with tile.TileContext(nc) as tc:
        with (
            tc.tile_pool(name="wpool", bufs=1) as wpool,
            tc.tile_pool(name="xpool", bufs=1) as xpool,
            tc.tile_pool(name="hpool", bufs=1) as hpool,
            tc.tile_pool(name="spool", bufs=1) as spool,
            tc.tile_pool(name="pspool", bufs=1, space="PSUM") as pspool,
        ):
            w = {}
            for name, ap in d_w.items():
                wt = wpool.tile(list(ap.shape), ap.dtype, name=f"w_{name}")
                nc.gpsimd.dma_start(wt[:], ap[:])
                w[name] = wt

            def PWk(gi, j4):
                k = gi * 4 + j4
                return w["PW"][:, k * 128:(k + 1) * 128]

            def XWq(name, q):
                return w[name][:, q * 128:(q + 1) * 128]

            for win in range(NG // W_GROUPS):
                Hd = {}
                Sd = {}
                NTd = {}
                xtb = {}
                for d in range(W_GROUPS // 2):
                    dbl = win * (W_GROUPS // 2) + d
                    ht = hpool.tile([128, 2, 512], BF16, tag=f"H{d}",
                                    bufs=2, name="ht")
                    nc.gpsimd.dma_start(ht[:], d_ht[dbl])
                    Hd[d] = ht
                psb = spool.tile([128, 6144], BF16, tag="psb", bufs=1,
                                 name="psb")
                pr = {}
                for t in range(T_FC):
                    if t % 4 == 0:
                        pr[0] = pspool.tile([128, 512], F32, tag="pp",
                                            bufs=2, name="pp")
                    pp = pr[0]
                    trh = w["TRH1"] if t else w["TRH0"]
                    tzh = w["TZH1"] if t else w["TZH0"]
                    q = t % XQ
                    xr = w["XR0P"] if t == 0 else XWq("XRP", q)
                    xz = w["XZ0P"] if t == 0 else XWq("XZP", q)
                    xi = w["XI0P"] if t == 0 else XWq("XIP", q)
                    def upd(d):
                        hm = spool.tile([128, 2, 512], BF16,
                                        tag=f"HM{d}", bufs=1, name="hm")
                        nc.vector.tensor_tensor(hm[:], Hd[d][:], NTd[d][:],
                                                op=ALU.subtract)
                        zt = spool.tile([128, 2, 512], BF16,
                                        tag=f"ZT{d}", bufs=1, name="zt")
                        nc.vector.tensor_tensor(zt[:], Sd[d][:, :, 512:1024],
                                                hm[:], op=ALU.mult)
                        nc.vector.tensor_tensor(Hd[d][:], NTd[d][:], zt[:],
                                                op=ALU.add)
                        for jj in (0, 1):
                            gidx = 2 * d + jj
                            nc.tensor.matmul(
                                pp[:], PWk(gidx, t % 4),
                                Hd[d][:, jj, :],
                                start=(t % 4 == 0 and gidx == 0),
                                stop=(t % 4 == 3 and gidx == 15),
                                skip_group_check=True)

                    pend = []
                    for gi in range(W_GROUPS):
                        d, j2 = gi // 2, gi % 2
                        dbl = win * (W_GROUPS // 2) + d
                        if t % XQ == 0 and j2 == 0:
                            xt_ = xpool.tile([128, 1024], BF16,
                                             tag=f"xt{d}", bufs=2, name="xt_")
                            nc.gpsimd.dma_start(
                                xt_[:], d_xtd[dbl, t // XQ])
                            xtb[d] = xt_
                        xts = xtb[d][:, j2 * 512:(j2 + 1) * 512]
                        hs = Hd[d][:, j2, :]

                        g = pspool.tile([128, 1024], F32, tag="gates",
                                        bufs=3, name="g")
                        nc.tensor.matmul(g[:, 0:512], trh[:], hs,
                                         start=True, stop=False)
                        nc.tensor.matmul(g[:, 0:512], xr[:], xts,
                                         start=False, stop=True)
                        nc.tensor.matmul(g[:, 512:1024], tzh[:], hs,
                                         start=True, stop=False)
                        nc.tensor.matmul(g[:, 512:1024], xz[:], xts,
                                         start=False, stop=True)

                        if j2 == 0:
                            Sd[d] = spool.tile([128, 2, 1024], BF16,
                                               tag=f"S{d}", bufs=1, name="S")
                            NTd[d] = spool.tile([128, 2, 512], BF16,
                                                tag=f"NT{d}", bufs=1,
                                                name="NT")
                        nc.scalar.activation(Sd[d][:, j2, :], g[:],
                                             AF.Sigmoid)

                        # phase 2: reuse gate banks for [i_n | h_n]
                        if t:
                            nc.tensor.matmul(g[:, 0:512], w["TIH1"][:], hs,
                                             start=True, stop=False)
                            nc.tensor.matmul(g[:, 0:512], xi[:], xts,
                                             start=False, stop=False)
                        else:
                            nc.tensor.matmul(g[:, 0:512], xi[:], xts,
                                             start=True, stop=False)
                        nc.tensor.matmul(g[:, 512:1024], w["THH"][:], hs,
                                         start=True, stop=True)

                        rhn = spool.tile([128, 512], BF16, tag="rhn",
                                         bufs=4, name="rhn")
                        nc.vector.scalar_tensor_tensor(
                            rhn[:], g[:, 512:1024], w["BHHN"][:],
                            Sd[d][:, j2, 0:512], op0=ALU.add, op1=ALU.mult)
                        nc.tensor.matmul(g[:, 0:512], w["ID128"][:], rhn[:],
                                         start=False, stop=True)
                        pend.append((d, j2, g))
                        if len(pend) > 1:
                            pd, pj, pg = pend.pop(0)
                            nc.scalar.activation(NTd[pd][:, pj, :],
                                                 pg[:, 0:512], AF.Tanh)
                            if pj == 1:
                                upd(pd)

                    while pend:
                        pd, pj, pg = pend.pop(0)
                        nc.scalar.activation(NTd[pd][:, pj, :],
                                             pg[:, 0:512], AF.Tanh)
                        if pj == 1:
                            upd(pd)

                    if t % 4 == 3:
                        blk = t // 4
                        nc.vector.tensor_copy(
                            psb[:, blk * 512:(blk + 1) * 512], pp[:])
                nc.gpsimd.dma_start(d_out[win], psb[:])

    nc.compile()
    _BUILT[key] = nc
    return nc


def _prep_weights(W_in, b_in, W_ih, W_hh, b_ih, b_hh, W_out, b_out):
    f8 = np.float64
    G = W_ih.astype(f8) @ W_in.astype(f8)              # [192, 4]
    c = W_ih.astype(f8) @ b_in.astype(f8) + b_ih       # [192]
    wo = W_out.astype(f8)[0]                           # [64]
    bo = float(b_out[0])
    Wr, Wz, Wn = W_hh[0:64].astype(f8), W_hh[64:128].astype(f8), \
        W_hh[128:192].astype(f8)
    Gr, Gz, Gn = G[0:64], G[64:128], G[128:192]
    cr, cz, cn = c[0:64], c[64:128], c[128:192]
    bhr, bhz, bhn = b_hh[0:64].astype(f8), b_hh[64:128].astype(f8), \
        b_hh[128:192].astype(f8)

    def blockdiag(m):  # [64,64] effective weight -> [128,128] lhsT
        out = np.zeros((128, 128), f8)
        out[0:64, 0:64] = m.T
        out[64:128, 64:128] = m.T
        return out

    def xlhs(Gg, bias):  # [9, 128] x-side lhsT
        out = np.zeros((9, 128), f8)
        out[0:3, 0:64] = Gg[:, 1:4].T
        out[3:6, 64:128] = Gg[:, 1:4].T
        out[6, 0:64] = bias
        out[6, 64:128] = bias
        out[7, 0:64] = Gg[:, 0]
        out[8, 64:128] = Gg[:, 0]
        return out

    def padq(x9, q):  # embed [9,128] lhsT at partition rows 9q:9q+9
        out = np.zeros((128, 128), np.float64)
        out[9 * q:9 * q + 9, :] = x9
        return out

    w = {}
    w["TRH0"] = blockdiag(Wr)
    w["TRH1"] = blockdiag(Wr + np.outer(Gr[:, 0], wo))
    w["TZH0"] = blockdiag(Wz)
    w["TZH1"] = blockdiag(Wz + np.outer(Gz[:, 0], wo))
    w["TIH1"] = blockdiag(np.outer(Gn[:, 0], wo))
    w["THH"] = blockdiag(Wn)
    w["ID128"] = np.eye(128, dtype=f8)
    w["XR0P"] = padq(xlhs(Gr, cr + bhr), 0)
    w["XZ0P"] = padq(xlhs(Gz, cz + bhz), 0)
    w["XI0P"] = padq(xlhs(Gn, cn), 0)
    xr1 = xlhs(Gr, cr + bhr + Gr[:, 0] * bo)
    xz1 = xlhs(Gz, cz + bhz + Gz[:, 0] * bo)
    xi1 = xlhs(Gn, cn + Gn[:, 0] * bo)
    w["XRP"] = np.concatenate([padq(xr1, q) for q in range(12)], axis=1)
    w["XZP"] = np.concatenate([padq(xz1, q) for q in range(12)], axis=1)
    w["XIP"] = np.concatenate([padq(xi1, q) for q in range(12)], axis=1)
    pw = np.zeros((128, 64 * 128), f8)
    for gi in range(16):
        for j4 in range(4):
            k = gi * 4 + j4
            col = 8 * gi + 2 * j4
            pw[0:64, k * 128 + col] = wo
            pw[64:128, k * 128 + col + 1] = wo
    w["PW"] = pw
    w["BHHN"] = np.concatenate([bhn, bhn])[:, None]

    import ml_dtypes
    return {k: np.ascontiguousarray(
        v.astype(np.float32 if k == "BHHN" else ml_dtypes.bfloat16))
        for k, v in w.items()}


def kernel(X, H, xn, W_in, b_in, W_ih, W_hh, b_ih, b_hh, W_out, b_out):
    global LAST_RESULTS
    import ml_dtypes
    X = np.asarray(X, np.float32)
    H = np.asarray(H, np.float32)
    xn = np.asarray(xn, np.float32)
    bo = float(np.asarray(b_out)[0])
    wmap = _prep_weights(np.asarray(W_in), np.asarray(b_in), np.asarray(W_ih),
                         np.asarray(W_hh), np.asarray(b_ih), np.asarray(b_hh),
                         np.asarray(W_out), np.asarray(b_out))

    Xs = X[:, T_HIST:T_HIST + T_FC, :, F_IN - 3:F_IN]  # [B, 48, C, 3]

    in_maps = []
    for ci in range(N_CORES):
        bs = slice(ci * B_LOC, (ci + 1) * B_LOC)
        # [t, f, col], col = b*C + c
        r3 = np.transpose(Xs[bs], (1, 3, 0, 2)).reshape(T_FC, 3, NCOLS)
        r6 = r3.reshape(T_FC, 3, ND, 2, 2, 512)  # [t,f,dbl,odd,ab,col]
        xtA = r6[:, :, :, :, 0, :].reshape(T_FC, 3, ND, 1024)
        xtB = r6[:, :, :, :, 1, :].reshape(T_FC, 3, ND, 1024)
        # rows 9q:9q+9 of block t//12 = [xtA(3); xtB(3); ones; xnA; xnB]
        XTD = np.zeros((ND, 4, 128, 1024), ml_dtypes.bfloat16)
        for t in range(T_FC):
            blk, qq = t // 12, t % 12
            XTD[:, blk, 9 * qq:9 * qq + 3] = xtA[t].transpose(1, 0, 2)
            XTD[:, blk, 9 * qq + 3:9 * qq + 6] = xtB[t].transpose(1, 0, 2)
            XTD[:, blk, 9 * qq + 6] = 1.0
        xn6 = xn[bs, :, 0].reshape(ND, 2, 2, 512)
        XTD[:, 0, 7, :] = xn6[:, :, 0, :].reshape(ND, 1024)
        XTD[:, 0, 8, :] = xn6[:, :, 1, :].reshape(ND, 1024)

        h6 = H[bs].reshape(NCOLS, HID).reshape(ND, 2, 2, 512, HID)
        HT = np.empty((ND, 128, 1024), ml_dtypes.bfloat16)
        HT[:, 0:64] = h6[:, :, 0].transpose(0, 3, 1, 2).reshape(ND, 64, 1024)
        HT[:, 64:128] = h6[:, :, 1].transpose(0, 3, 1, 2) \
            .reshape(ND, 64, 1024)

        m = {"XTD": XTD, "HT": np.ascontiguousarray(HT)}
        m.update(wmap)
        in_maps.append(m)

    nc = _build()

    trace = os.environ.get("BASS_KERNEL_TRACE") == "1"
    if trace:
        _register_ntff_hook()
    res = run_bass_kernel_spmd(nc, in_maps, list(range(N_CORES)), trace=trace)
    LAST_RESULTS = res

    out = np.empty((B, T_FC, C, 1), np.float32)
    for ci in range(N_CORES):
        O = np.asarray(res.results[ci]["OUT"], np.float32)
        # rows: 8*gi + 2*j4 + ch; cols: 12 blocks of 512
        O7 = O.reshape(2, 16, 4, 2, 12, 512)
        # [w, gi, j4, ch, blk, col] -> [blk, j4, w, gi, ch, col] = [t, row]
        P = O7.transpose(4, 2, 0, 1, 3, 5).reshape(T_FC, NCOLS) + bo
        out[ci * B_LOC:(ci + 1) * B_LOC] = \
            P.reshape(T_FC, B_LOC, C).transpose(1, 0, 2)[..., None]
    return out


def _register_ntff_hook():
    """The agent image's antenv lacks axon_hooks; provide it so trace=True
    can capture NTFF profiles through libaxon_pjrt."""
    import sys
    import types
    if "antenv.axon_hooks" in sys.modules:
        return
    mod = types.ModuleType("antenv.axon_hooks")
    state = {"hook": None}
    mod.set_axon_ntff_profile_hook = lambda h: state.update(hook=h)
    mod.get_axon_ntff_profile_hook = lambda: state["hook"]
    sys.modules["antenv.axon_hooks"] = mod
    try:
        import antenv
        antenv.axon_hooks = mod
    except ImportError:
        pass
    try:
        from trn_agent_boot.trn_boot import _ntff_profile_via_ctypes
        hook = _ntff_profile_via_ctypes("/opt/axon/libaxon_pjrt.so")
        if hook is not None:
            mod.set_axon_ntff_profile_hook(hook)
    except Exception as e:  # pragma: no cover
        print(f"NTFF hook registration failed: {e}")
    # No artifact bucket in this sandbox; keep profiles local.
    import concourse.bass_utils as bu
    bu.upload_artifacts = lambda tmpdir: f"file://{tmpdir}"


# revision 22
# speedup vs baseline: 1.0018x; 1.0018x over previous
"""Trainium2 Bass kernel for nn_Decoder (GRU decoder, B=64, T_FC=48, C=4096, HID=64).

Strategy (v2)
-------------
Data-parallel over batch: 8 cores x 8 batch rows -> 32768 independent GRU
"columns" per core.  Columns are processed in GROUPS of 1024: chunk A
(cols 0:512) occupies partitions 0:64, chunk B (cols 512:1024) partitions
64:128, so every DVE/ACT op runs with all 128 partitions busy.

Host algebra folds fc_in and the autoregressive x_prev feedback into the
gate weights (t>=1):
    G = W_ih @ W_in                       [192, 4]
    pre_g   = (W_hh_g + G_g0 wo^T) h + G_g,1:4 xt + bias_g     (g in r,z)
    i_n     = (G_n0 wo^T) h + G_n,1:4 xt + bias_n
    h_n     = W_hh_n h            (+ b_hh_n via scalar_tensor_tensor)
    n = tanh(i_n + r*h_n);  h' = n + z*(h - n);  pred = wo @ h' (+ b_out host)

Per group-step: 9 back-to-back N=512 bf16 matmuls -- ALL K=128 (any
K<128 matmul runs ~2x slower on this HW and poisons its neighbours, so
the K=9 x-side lhsTs are embedded at partition rows 9q:9q+9 of a
[128,128] zero-padded weight, against a [128,1024] tile that packs 12
timesteps of xt data in the partition dim).  Block-diagonal K=128
h-side matmuls; an identity-matmul accumulates r*h_n into the i_n psum
(gate banks are reused phase1->phase2); a sparse-lhsT pred matmul parks
4 steps x 8 groups of predictions in one double-buffered psum bank.
Per group-step ACT: 1 sigmoid [128,1024] + 1 tanh [128,512]; DVE: one
fused scalar_tensor_tensor (b_hh_n add + r mult) and 3 tensor_tensor
update ops at [128,1024] per double-group.  Final balance: PE 92%,
ACT 99%, DVE 79% busy.
"""

import os

import numpy as np

import concourse.bass as bass
import concourse.mybir as mybir
import concourse.tile as tile
from concourse import bacc
from concourse.bass_utils import run_bass_kernel_spmd

F32 = mybir.dt.float32
BF16 = mybir.dt.bfloat16
AF = mybir.ActivationFunctionType
ALU = mybir.AluOpType

B, T_HIST, T_FC, C, F_IN, HID = 64, 24, 48, 4096, 8, 64
N_CORES = 8
B_LOC = B // N_CORES
NCOLS = B_LOC * C          # 32768 columns per core
NG = 32                    # groups of 1024 columns
ND = 16                    # double-groups
W_GROUPS = 8               # groups per window (4 double-groups)
XQ = 12                    # xt steps packed per [128,1024] tile

_BUILT = {}
LAST_RESULTS = None  # BassKernelResults of the most recent run (for test.py)

W_SHAPES = {
    "TRH0": [128, 128], "TRH1": [128, 128],
    "TZH0": [128, 128], "TZH1": [128, 128],
    "TIH1": [128, 128], "THH": [128, 128], "ID128": [128, 128],
    "XR0P": [128, 128], "XZ0P": [128, 128], "XI0P": [128, 128],
    "XRP": [128, 12 * 128], "XZP": [128, 12 * 128], "XIP": [128, 12 * 128],
    "PW": [128, 32 * 128],
    "BHHN": [128, 1],
}


def _build():
    key = "v2"
    if key in _BUILT:
        return _BUILT[key]

    nc = bacc.Bacc("TRN2", target_bir_lowering=False, debug=False,
                   num_devices=N_CORES)

    d_xtd = nc.dram_tensor("XTD", [ND, 4, 128, 1024], BF16,
                           kind="ExternalInput").ap()
    d_ht = nc.dram_tensor("HT", [ND, 128, 1024], BF16,
                          kind="ExternalInput").ap()
    d_w = {name: nc.dram_tensor(name, shape,
                                F32 if name == "BHHN" else BF16,
                                kind="ExternalInput").ap()
           for name, shape in W_SHAPES.items()}
    # preds: [window, 128, 12*512]; row = 16*gi + 2*(t%4) + chunk
    d_out = nc.dram_tensor("OUT", [NG // W_GROUPS, 128, 6144], BF16,
                           kind="ExternalOutput").ap()

    with tile.TileContext(nc) as tc:
        with (
            tc.tile_pool(name="wpool", bufs=1) as wpool,
            tc.tile_pool(name="xpool", bufs=1) as xpool,
            tc.tile_pool(name="hpool", bufs=1) as hpool,
            tc.tile_pool(name="spool", bufs=1) as spool,
            tc.tile_pool(name="pspool", bufs=1, space="PSUM") as pspool,
        ):
            w = {}
            for name, ap in d_w.items():
                wt = wpool.tile(list(ap.shape), ap.dtype, name=f"w_{name}")
                nc.gpsimd.dma_start(wt[:], ap[:])
                w[name] = wt

            def PWk(gi, j4):
                k = gi * 4 + j4
                return w["PW"][:, k * 128:(k + 1) * 128]

            def XWq(name, q):
                return w[name][:, q * 128:(q + 1) * 128]

            for win in range(NG // W_GROUPS):
                Hd = {}
                Sd = {}
                NTd = {}
                xtb = {}
                for d in range(4):
                    dbl = win * 4 + d
                    ht = hpool.tile([128, 2, 512], BF16, tag=f"H{d}",
                                    bufs=2, name="ht")
                    nc.gpsimd.dma_start(ht[:], d_ht[dbl])
                    Hd[d] = ht
                psb = spool.tile([128, 6144], BF16, tag="psb", bufs=2,
                                 name="psb")
                pr = {}
                for t in range(T_FC):
                    if t % 4 == 0:
                        pr[0] = pspool.tile([128, 512], F32, tag="pp",
                                            bufs=2, name="pp")
                    pp = pr[0]
                    trh = w["TRH1"] if t else w["TRH0"]
                    tzh = w["TZH1"] if t else w["TZH0"]
                    q = t % XQ
                    xr = w["XR0P"] if t == 0 else XWq("XRP", q)
                    xz = w["XZ0P"] if t == 0 else XWq("XZP", q)
                    xi = w["XI0P"] if t == 0 else XWq("XIP", q)
                    def upd(d):
                        hm = spool.tile([128, 2, 512], BF16,
                                        tag=f"HM{d}", bufs=3, name="hm")
                        nc.vector.tensor_tensor(hm[:], Hd[d][:], NTd[d][:],
                                                op=ALU.subtract)
                        zt = spool.tile([128, 2, 512], BF16,
                                        tag=f"ZT{d}", bufs=3, name="zt")
                        nc.vector.tensor_tensor(zt[:], Sd[d][:, :, 512:1024],
                                                hm[:], op=ALU.mult)
                        nc.vector.tensor_tensor(Hd[d][:], NTd[d][:], zt[:],
                                                op=ALU.add)
                        for jj in (0, 1):
                            gidx = 2 * d + jj
                            nc.tensor.matmul(
                                pp[:], PWk(gidx, t % 4),
                                Hd[d][:, jj, :],
                                start=(t % 4 == 0 and gidx == 0),
                                stop=(t % 4 == 3 and gidx == 7),
                                skip_group_check=True)

                    pend = []
                    for gi in range(W_GROUPS):
                        d, j2 = gi // 2, gi % 2
                        dbl = win * 4 + d
                        if t % XQ == 0 and j2 == 0:
                            xt_ = xpool.tile([128, 1024], BF16,
                                             tag=f"xt{d}", bufs=2, name="xt_")
                            nc.gpsimd.dma_start(
                                xt_[:], d_xtd[dbl, t // XQ])
                            xtb[d] = xt_
                        xts = xtb[d][:, j2 * 512:(j2 + 1) * 512]
                        hs = Hd[d][:, j2, :]

                        g = pspool.tile([128, 1024], F32, tag="gates",
                                        bufs=3, name="g")
                        nc.tensor.matmul(g[:, 0:512], trh[:], hs,
                                         start=True, stop=False)
                        nc.tensor.matmul(g[:, 0:512], xr[:], xts,
                                         start=False, stop=True)
                        nc.tensor.matmul(g[:, 512:1024], tzh[:], hs,
                                         start=True, stop=False)
                        nc.tensor.matmul(g[:, 512:1024], xz[:], xts,
                                         start=False, stop=True)

                        if j2 == 0:
                            Sd[d] = spool.tile([128, 2, 1024], BF16,
                                               tag=f"S{d}", bufs=3, name="S")
                            NTd[d] = spool.tile([128, 2, 512], BF16,
                                                tag=f"NT{d}", bufs=3,
                                                name="NT")
                        nc.scalar.activation(Sd[d][:, j2, :], g[:],
                                             AF.Sigmoid)

                        # phase 2: reuse gate banks for [i_n | h_n]
                        if t:
                            nc.tensor.matmul(g[:, 0:512], w["TIH1"][:], hs,
                                             start=True, stop=False)
                            nc.tensor.matmul(g[:, 0:512], xi[:], xts,
                                             start=False, stop=False)
                        else:
                            nc.tensor.matmul(g[:, 0:512], xi[:], xts,
                                             start=True, stop=False)
                        nc.tensor.matmul(g[:, 512:1024], w["THH"][:], hs,
                                         start=True, stop=True)

                        rhn = spool.tile([128, 512], BF16, tag="rhn",
                                         bufs=4, name="rhn")
                        nc.vector.scalar_tensor_tensor(
                            rhn[:], g[:, 512:1024], w["BHHN"][:],
                            Sd[d][:, j2, 0:512], op0=ALU.add, op1=ALU.mult)
                        nc.tensor.matmul(g[:, 0:512], w["ID128"][:], rhn[:],
                                         start=False, stop=True)
                        pend.append((d, j2, g))
                        if len(pend) > 1:
                            pd, pj, pg = pend.pop(0)
                            nc.scalar.activation(NTd[pd][:, pj, :],
                                                 pg[:, 0:512], AF.Tanh)
                            if pj == 1:
                                upd(pd)

                    while pend:
                        pd, pj, pg = pend.pop(0)
                        nc.scalar.activation(NTd[pd][:, pj, :],
                                             pg[:, 0:512], AF.Tanh)
                        if pj == 1:
                            upd(pd)

                    if t % 4 == 3:
                        blk = t // 4
                        nc.vector.tensor_copy(
                            psb[:, blk * 512:(blk + 1) * 512], pp[:])
                nc.gpsimd.dma_start(d_out[win], psb[:])

    nc.compile()
    _BUILT[key] = nc
    return nc


def _prep_weights(W_in, b_in, W_ih, W_hh, b_ih, b_hh, W_out, b_out):
    f8 = np.float64
    G = W_ih.astype(f8) @ W_in.astype(f8)              # [192, 4]
    c = W_ih.astype(f8) @ b_in.astype(f8) + b_ih       # [192]
    wo = W_out.astype(f8)[0]                           # [64]
    bo = float(b_out[0])
    Wr, Wz, Wn = W_hh[0:64].astype(f8), W_hh[64:128].astype(f8), \
        W_hh[128:192].astype(f8)
    Gr, Gz, Gn = G[0:64], G[64:128], G[128:192]
    cr, cz, cn = c[0:64], c[64:128], c[128:192]
    bhr, bhz, bhn = b_hh[0:64].astype(f8), b_hh[64:128].astype(f8), \
        b_hh[128:192].astype(f8)

    def blockdiag(m):  # [64,64] effective weight -> [128,128] lhsT
        out = np.zeros((128, 128), f8)
        out[0:64, 0:64] = m.T
        out[64:128, 64:128] = m.T
        return out

    def xlhs(Gg, bias):  # [9, 128] x-side lhsT
        out = np.zeros((9, 128), f8)
        out[0:3, 0:64] = Gg[:, 1:4].T
        out[3:6, 64:128] = Gg[:, 1:4].T
        out[6, 0:64] = bias
        out[6, 64:128] = bias
        out[7, 0:64] = Gg[:, 0]
        out[8, 64:128] = Gg[:, 0]
        return out

    def padq(x9, q):  # embed [9,128] lhsT at partition rows 9q:9q+9
        out = np.zeros((128, 128), np.float64)
        out[9 * q:9 * q + 9, :] = x9
        return out

    w = {}
    w["TRH0"] = blockdiag(Wr)
    w["TRH1"] = blockdiag(Wr + np.outer(Gr[:, 0], wo))
    w["TZH0"] = blockdiag(Wz)
    w["TZH1"] = blockdiag(Wz + np.outer(Gz[:, 0], wo))
    w["TIH1"] = blockdiag(np.outer(Gn[:, 0], wo))
    w["THH"] = blockdiag(Wn)
    w["ID128"] = np.eye(128, dtype=f8)
    w["XR0P"] = padq(xlhs(Gr, cr + bhr), 0)
    w["XZ0P"] = padq(xlhs(Gz, cz + bhz), 0)
    w["XI0P"] = padq(xlhs(Gn, cn), 0)
    xr1 = xlhs(Gr, cr + bhr + Gr[:, 0] * bo)
    xz1 = xlhs(Gz, cz + bhz + Gz[:, 0] * bo)
    xi1 = xlhs(Gn, cn + Gn[:, 0] * bo)
    w["XRP"] = np.concatenate([padq(xr1, q) for q in range(12)], axis=1)
    w["XZP"] = np.concatenate([padq(xz1, q) for q in range(12)], axis=1)
    w["XIP"] = np.concatenate([padq(xi1, q) for q in range(12)], axis=1)
    pw = np.zeros((128, 32 * 128), f8)
    for gi in range(8):
        for j4 in range(4):
            k = gi * 4 + j4
            col = 16 * gi + 2 * j4
            pw[0:64, k * 128 + col] = wo
            pw[64:128, k * 128 + col + 1] = wo
    w["PW"] = pw
    w["BHHN"] = np.concatenate([bhn, bhn])[:, None]

    import ml_dtypes
    return {k: np.ascontiguousarray(
        v.astype(np.float32 if k == "BHHN" else ml_dtypes.bfloat16))
        for k, v in w.items()}


def kernel(X, H, xn, W_in, b_in, W_ih, W_hh, b_ih, b_hh, W_out, b_out):
    global LAST_RESULTS
    import ml_dtypes
    X = np.asarray(X, np.float32)
    H = np.asarray(H, np.float32)
    xn = np.asarray(xn, np.float32)
    bo = float(np.asarray(b_out)[0])
    wmap = _prep_weights(np.asarray(W_in), np.asarray(b_in), np.asarray(W_ih),
                         np.asarray(W_hh), np.asarray(b_ih), np.asarray(b_hh),
                         np.asarray(W_out), np.asarray(b_out))

    Xs = X[:, T_HIST:T_HIST + T_FC, :, F_IN - 3:F_IN]  # [B, 48, C, 3]

    in_maps = []
    for ci in range(N_CORES):
        bs = slice(ci * B_LOC, (ci + 1) * B_LOC)
        # [t, f, col], col = b*C + c
        r3 = np.transpose(Xs[bs], (1, 3, 0, 2)).reshape(T_FC, 3, NCOLS)
        r6 = r3.reshape(T_FC, 3, ND, 2, 2, 512)  # [t,f,dbl,odd,ab,col]
        xtA = r6[:, :, :, :, 0, :].reshape(T_FC, 3, ND, 1024)
        xtB = r6[:, :, :, :, 1, :].reshape(T_FC, 3, ND, 1024)
        # rows 9q:9q+9 of block t//12 = [xtA(3); xtB(3); ones; xnA; xnB]
        XTD = np.zeros((ND, 4, 128, 1024), ml_dtypes.bfloat16)
        for t in range(T_FC):
            blk, qq = t // 12, t % 12
            XTD[:, blk, 9 * qq:9 * qq + 3] = xtA[t].transpose(1, 0, 2)
            XTD[:, blk, 9 * qq + 3:9 * qq + 6] = xtB[t].transpose(1, 0, 2)
            XTD[:, blk, 9 * qq + 6] = 1.0
        xn6 = xn[bs, :, 0].reshape(ND, 2, 2, 512)
        XTD[:, 0, 7, :] = xn6[:, :, 0, :].reshape(ND, 1024)
        XTD[:, 0, 8, :] = xn6[:, :, 1, :].reshape(ND, 1024)

        h6 = H[bs].reshape(NCOLS, HID).reshape(ND, 2, 2, 512, HID)
        HT = np.empty((ND, 128, 1024), ml_dtypes.bfloat16)
        HT[:, 0:64] = h6[:, :, 0].transpose(0, 3, 1, 2).reshape(ND, 64, 1024)
        HT[:, 64:128] = h6[:, :, 1].transpose(0, 3, 1, 2) \
            .reshape(ND, 64, 1024)

        m = {"XTD": XTD, "HT": np.ascontiguousarray(HT)}
        m.update(wmap)
        in_maps.append(m)

    nc = _build()

    trace = os.environ.get("BASS_KERNEL_TRACE") == "1"
    if trace:
        _register_ntff_hook()
    res = run_bass_kernel_spmd(nc, in_maps, list(range(N_CORES)), trace=trace)
    LAST_RESULTS = res

    out = np.empty((B, T_FC, C, 1), np.float32)
    for ci in range(N_CORES):
        O = np.asarray(res.results[ci]["OUT"], np.float32)
        # rows: 16*gi + 2*j4 + ch (q<8); cols: 12 blocks of 512
        O7 = O.reshape(4, 8, 16, 12, 512)[:, :, 0:8] \
            .reshape(4, 8, 4, 2, 12, 512)
        # [w, gi, j4, ch, blk, col] -> [blk, j4, w, gi, ch, col] = [t, row]
        P = O7.transpose(4, 2, 0, 1, 3, 5).reshape(T_FC, NCOLS) + bo
        out[ci * B_LOC:(ci + 1) * B_LOC] = \
            P.reshape(T_FC, B_LOC, C).transpose(1, 0, 2)[..., None]
    return out


def _register_ntff_hook():
    """The agent image's antenv lacks axon_hooks; provide it so trace=True
    can capture NTFF profiles through libaxon_pjrt."""
    import sys
    import types
    if "antenv.axon_hooks" in sys.modules:
        return
    mod = types.ModuleType("antenv.axon_hooks")
    state = {"hook": None}
    mod.set_axon_ntff_profile_hook = lambda h: state.update(hook=h)
    mod.get_axon_ntff_profile_hook = lambda: state["hook"]
    sys.modules["antenv.axon_hooks"] = mod
    try:
        import antenv
        antenv.axon_hooks = mod
    except ImportError:
        pass
    try:
        from trn_agent_boot.trn_boot import _ntff_profile_via_ctypes
        hook = _ntff_profile_via_ctypes("/opt/axon/libaxon_pjrt.so")
        if hook is not None:
            mod.set_axon_ntff_profile_hook(hook)
    except Exception as e:  # pragma: no cover
        print(f"NTFF hook registration failed: {e}")
    # No artifact bucket in this sandbox; keep profiles local.
    import concourse.bass_utils as bu
    bu.upload_artifacts = lambda tmpdir: f"file://{tmpdir}"
